# revision 17
# baseline (speedup 1.0000x reference)
"""BrushStroke splat kernel for 8 trn2 NeuronCores.

out[b,c,y,x] = mean_n sum_{p,q} Fy[b,n,y,p] Fx[b,n,x,q] patches[b,n,c,p,q]
with Fx/Fy separable Gaussian filter banks (sigma=0.1) normalized over a
padded spatial axis.

Strategy (per core, 2 batches of 64 strokes), v2 — no DMA gathers:
 - Filter tiles are computed on-chip per group of 4 strokes: one
   Derivative_Erf activation per axis evaluates
   (2/sqrt(pi)) * exp(-((t + q') - (g + 31.5))^2 / (2 sigma^2))
   directly on a [128(j,q'), 288] iota tile using a per-partition bias
   built once in the preamble via a tiny one-hot matmul (the 2/sqrt(pi)
   and 1/64 factors cancel into the row-sum normalizers).
 - MM1 per group: 3 matmuls (one per channel) with a block-diagonal
   bf16 lhsT of the 4 strokes' patch blocks -> t in one PSUM span
   [128, 768]; a single Pool-engine drain rescales by 1/sum(Fy) into a
   bf16 [128, 768] SBUF tile.
 - MM2 per group: 4 matmuls (2 y-tiles x {c0c1 merged, c2}) chained over
   the 16 groups into 3 PSUM accumulators; unnormalized Fy rows serve as
   the stationary directly.
 - MM2 for group g is emitted one iteration late so the tensor engine
   always has the next group's MM1 available (no drain-latency bubble).
Batch-parallel across cores; no collectives.
"""
import sys, types
import numpy as np

IMAGE = 256
PAD = 16
EPS = 1e-7
SIGMA2 = 2.0 * 0.1 ** 2
B, N, C, PH, PW = 16, 64, 3, 32, 32
NCORES = 8
BLOC = B // NCORES          # 2 batches per core
NG = N // 4                 # 16 groups of 4 strokes
W288 = IMAGE + 2 * PAD      # padded spatial axis length
SCL = (1.0 / SIGMA2) ** 0.5  # derf(SCL*t + SCL*b) ~ exp(-(t+b)^2/SIGMA2)
CX = PW / 2 - 0.5 + PAD      # 31.5
CY = PW / 2 - 0.4 + PAD      # 31.6


def _install_patches():
    if 'antenv.axon_hooks' not in sys.modules:
        mod = types.ModuleType('antenv.axon_hooks')
        mod._hook = None
        mod.set_axon_ntff_profile_hook = lambda h: setattr(mod, '_hook', h)
        mod.get_axon_ntff_profile_hook = lambda: mod._hook
        sys.modules['antenv.axon_hooks'] = mod
        try:
            from trn_agent_boot.trn_boot import _ntff_profile_via_ctypes
            hook = _ntff_profile_via_ctypes('/opt/axon/libaxon_pjrt.so')
            if hook is not None:
                mod.set_axon_ntff_profile_hook(hook)
        except Exception:
            pass

    import concourse.tile as tile
    import concourse.bass_utils as bass_utils
    from concourse.vector_clock import ScopedClock

    bass_utils.upload_artifacts = lambda tmpdir: 'local://' + tmpdir

    if getattr(tile.TileContext._drain_and_barrier, '_patched', False):
        return

    def _drain_and_barrier(self, tick_clock, wait_clock):
        nc = self.nc
        drain_inst = nc.sync.drain()
        wait_clock.add_sem_waits(
            drain_inst.ins, ScopedClock({None: tick_clock.global_clock}))
        si = drain_inst.ins.sync_info
        waits = list(si.on_wait or [])
        si.on_wait = []
        for w in waits:
            nop = nc.sync.nop()
            nop.ins.sync_info = type(si)(on_wait=[w], on_update=[])
        nc.all_engine_barrier()
        popped = nc._tile_sem_poison_stack.pop()
        assert popped is self._sem_poison
        nc.clear_and_free_semaphores(list(self.sems.allocated().values()))
        nc.all_engine_barrier()

    _drain_and_barrier._patched = True
    tile.TileContext._drain_and_barrier = _drain_and_barrier


def _split_multi_waits(nc):
    """This walrus accepts at most one sync wait per instruction; hoist
    extras onto same-engine NoOps inserted just before."""
    import bass_rust
    n_new = [0]

    def fresh_nop(engine, wait, si_type):
        n_new[0] += 1
        nop = bass_rust.InstNoOp(name=f'I-waitsplit-{n_new[0]}', ins=[], outs=[])
        nop.engine = engine
        nop.sync_info = si_type(on_wait=[wait], on_update=[])
        return nop

    for fn in nc.m.functions:
        for blk in fn.blocks:
            insts = blk.instructions
            i = 0
            while i < len(insts):
                inst = insts[i]
                si = inst.sync_info
                if si is not None and si.on_wait and len(si.on_wait) > 1:
                    waits = list(si.on_wait)
                    si.on_wait = [waits[-1]]
                    for k, w in enumerate(waits[:-1]):
                        insts.insert(i + k, fresh_nop(inst.engine, w, type(si)))
                    i += len(waits) - 1
                i += 1


_PROGRAM = None


def _build_program():
    global _PROGRAM
    if _PROGRAM is not None:
        return _PROGRAM
    _install_patches()
    import concourse.bass as bass
    import concourse.tile as tile
    from concourse import mybir
    from bass_rust import AP

    f32 = mybir.dt.float32
    bf16 = mybir.dt.bfloat16
    AF = mybir.ActivationFunctionType
    AX = mybir.AxisListType
    MUL, SUB = mybir.AluOpType.mult, mybir.AluOpType.subtract

    nc = bass.Bass('TRN2', target_bir_lowering=False, debug=False,
                   num_devices=NCORES)
    # inputs (per core): brush coords by (batch,coord) rows; patches in
    # reversed-(p,q) block layout (bf16); one-hot masks for the bias
    # broadcast matmul; 4x4 identity for the tiny transpose
    g_in = nc.declare_dram_parameter('g_in', [4, N], f32, isOutput=False)
    pt_in = nc.declare_dram_parameter('pt_in', [BLOC, 128, NG * C * PH], bf16,
                                      isOutput=False)
    id4 = nc.declare_dram_parameter('id4', [4, 4], f32, isOutput=False)
    mask4 = nc.declare_dram_parameter('mask4', [N, 128], f32, isOutput=False)
    mask16 = nc.declare_dram_parameter('mask16', [N, NG], f32, isOutput=False)
    qp_in = nc.declare_dram_parameter('qp_in', [128, 1], f32, isOutput=False)
    y_out = nc.declare_dram_parameter('y_out', [BLOC, C, IMAGE, IMAGE], f32,
                                      isOutput=True)

    with tile.TileContext(nc) as tc:
        with tc.tile_pool(name='glob', bufs=1) as gp, \
             tc.tile_pool(name='work', bufs=1) as wp, \
             tc.tile_pool(name='ps1', bufs=2, space='PSUM') as ps1, \
             tc.tile_pool(name='ps2', bufs=1, space='PSUM') as ps2:
            # accumulators: one PSUM bank per chain (interleaved accumulation
            # groups sharing a bank corrupt each other); a2t[0]'s bank also
            # hosts the tiny preamble matmul/transpose outputs
            a01t = [ps2.tile([128, 512], f32, name=f'a01_{yt}')
                    for yt in range(2)]
            a2t = [ps2.tile([128, 256], f32, name=f'a2_{yt}')
                   for yt in range(2)]
            # ---- input DMAs, spread across queues ----
            bc = gp.tile([4, N], f32)
            nc.sync.dma_start(bc[:], g_in[:])
            idt = gp.tile([4, 4], f32)
            nc.sync.dma_start(idt[:], id4[:])
            m4 = gp.tile([N, 128], f32)
            nc.scalar.dma_start(m4[:], mask4[:])
            m16 = gp.tile([N, NG], f32)
            nc.scalar.dma_start(m16[:], mask16[:])
            qp = gp.tile([128, 1], f32)
            nc.scalar.dma_start(qp[:], qp_in[:])
            ptc = []
            for b in range(BLOC):
                t_ = gp.tile([128, NG * C * PH], bf16, name=f'ptc{b}')
                [nc.gpsimd, nc.sync][b].dma_start(t_[:], pt_in[b])
                ptc.append(t_)

            # ---- iota ramp (shared by all activations) ----
            it = gp.tile([128, W288], f32)
            nc.gpsimd.iota(it[:], pattern=[[1, W288]], base=0,
                           channel_multiplier=0,
                           allow_small_or_imprecise_dtypes=True)

            # ---- block-diagonal patch lhsT, batch 0 (zeros + 4 copies) ----
            ps_all = [gp.tile([128, 128 * C * NG], bf16, name=f'psall{b}')
                      for b in range(BLOC)]

            def emit_psall_memset(b, lo, hi, eng):
                v = ps_all[b].bitcast(f32)
                eng.memset(v[:, lo:hi], 0.0)

            def emit_psall_copy(b, j, eng):
                dst0 = ps_all[b][32 * j:32 * j + 1, 32 * j:32 * j + 1]
                dst = AP(ps_all[b].tensor, dst0.offset,
                         [[128 * C * NG, 32], [128 * C, NG],
                          [128, C], [1, PH]])
                src0 = ptc[b][32 * j:32 * j + 1, 0:1]
                srcap = AP(ptc[b].tensor, src0.offset,
                           [[NG * C * PH, 32], [C * PH, NG],
                            [PH, C], [1, PH]])
                eng.tensor_copy(dst, srcap)

            emit_psall_memset(0, 0, 1536, nc.vector)
            emit_psall_memset(0, 1536, 3072, nc.gpsimd)

            # ---- brush normalization -> per-group bias matrix ----
            mn = gp.tile([4, 1], f32)
            mx = gp.tile([4, 1], f32)
            nc.vector.tensor_reduce(mn[:], bc[:], axis=AX.X,
                                    op=mybir.AluOpType.min)
            nc.vector.reduce_max(mx[:], bc[:], axis=AX.X)
            rng = gp.tile([4, 1], f32)
            nc.vector.tensor_sub(rng[:], mx[:], mn[:])
            nc.vector.tensor_scalar_add(rng[:], rng[:], EPS)
            inv = gp.tile([4, 1], f32)
            nc.vector.reciprocal(inv[:], rng[:])
            nc.vector.tensor_scalar_mul(inv[:], inv[:], float(IMAGE))
            gn = gp.tile([4, N], f32)
            nc.vector.tensor_scalar_sub(gn[:], bc[:], mn[:])
            nc.vector.tensor_scalar_mul(gn[:], gn[:], inv[:])

            # preamble PSUM lives in a2t[0]'s bank: psB in [:, 0:64],
            # transpose in [0:64, 64:68]
            pre = a2t[0]
            # transpose [4,N] -> [N,4] (cols: b0x, b0y, b1x, b1y)
            tp_ps = pre[0:N, 64:68]
            nc.tensor.transpose(tp_ps, gn[:], idt[:])
            tp = gp.tile([N, 4], f32)
            nc.vector.tensor_copy(tp[:], tp_ps)

            # negC4[n, k] = -(g_n + C{X,Y});  rhsA = mask16 * negC4
            negC4 = gp.tile([N, 4], f32)
            for k in range(4):
                CC = CX if k % 2 == 0 else CY
                nc.vector.tensor_scalar(negC4[:, k:k + 1], tp[:, k:k + 1],
                                        -1.0, CC, MUL, SUB)
            rhsA = gp.tile([N, 64], f32)
            for k in range(4):
                nc.vector.tensor_scalar_mul(rhsA[:, 16 * k:16 * (k + 1)],
                                            m16[:], negC4[:, k:k + 1])
            # BiasAll[32j+q', 16k+g] = q' - (g_{4g+j,k} + C)
            psB = pre[:, 0:64]
            nc.tensor.matmul(psB, m4[:], rhsA[:], start=True, stop=True)
            BiasAll = gp.tile([128, 64], f32)
            nc.vector.tensor_scalar_add(BiasAll[:], psB, qp[:])

            emit_psall_copy(0, 0, nc.vector)
            emit_psall_copy(0, 1, nc.gpsimd)
            emit_psall_copy(0, 2, nc.vector)
            emit_psall_copy(0, 3, nc.gpsimd)

            # ---- main loop: one-group software pipeline ----
            prev = None
            NTOT = BLOC * NG
            for k in range(NTOT + 1):
                if k < NTOT:
                    b, g = divmod(k, NG)
                    colx, coly = 32 * b + g, 32 * b + 16 + g
                    # u = [iota + bias_x | iota + bias_y] (Pool, SBUF only)
                    u = wp.tile([128, 2 * W288], f32, name='u', tag='u',
                                bufs=2)
                    nc.gpsimd.tensor_scalar_add(u[:, 0:W288], it[:],
                                                BiasAll[:, colx:colx + 1])
                    nc.gpsimd.tensor_scalar_add(u[:, W288:2 * W288], it[:],
                                                BiasAll[:, coly:coly + 1])
                    # one activation: (2/sqrt(pi)) exp(-(SCL*u)^2)
                    fxy = wp.tile([128, 2 * W288], bf16, name='fxy',
                                  tag='fxy', bufs=3)
                    nc.scalar.activation(fxy[:], u[:], AF.Derivative_Erf,
                                         bias=0.0, scale=SCL)
                    # row sums over both padded windows -> 1/(N*Sx), 1/Sy
                    s2 = wp.tile([128, 2], f32, name='s2', tag='s2', bufs=2)
                    nc.vector.reduce_sum(s2[:, 0:1], fxy[:, 0:W288],
                                         axis=AX.X)
                    nc.vector.reduce_sum(s2[:, 1:2], fxy[:, W288:2 * W288],
                                         axis=AX.X)
                    nc.gpsimd.tensor_scalar_mul(s2[:, 0:1], s2[:, 0:1],
                                                float(N))
                    iv2 = wp.tile([128, 2], f32, name='iv2', tag='iv2',
                                  bufs=3)
                    nc.vector.reciprocal(iv2[:], s2[:])
                    # normalized filters (Pool)
                    fxn = wp.tile([128, IMAGE], bf16, name='fxn', tag='fxn',
                                  bufs=2)
                    nc.gpsimd.tensor_scalar_mul(fxn[:],
                                                fxy[:, PAD:PAD + IMAGE],
                                                iv2[:, 0:1])
                    fyn = wp.tile([128, IMAGE], bf16, name='fyn', tag='fyn',
                                  bufs=3)
                    nc.gpsimd.tensor_scalar_mul(
                        fyn[:], fxy[:, W288 + PAD:W288 + PAD + IMAGE],
                        iv2[:, 1:2])
                    # MM1: 3 channels into one PSUM span
                    pfull = ps1.tile([128, 768], f32, name='pfull',
                                     tag='pfull')
                    for c in range(C):
                        nc.tensor.matmul(
                            pfull[:, 256 * c:256 * (c + 1)],
                            ps_all[b][:, 384 * g + 128 * c:
                                      384 * g + 128 * (c + 1)],
                            fxn[:], start=True, stop=True)
                    # PSUM->SBUF drain split across DVE and ACT
                    tall = wp.tile([128, 768], bf16, name='tall', tag='tall',
                                   bufs=2)
                    nc.vector.tensor_copy(tall[:, 0:480], pfull[:, 0:480])
                    nc.scalar.copy(tall[:, 480:768], pfull[:, 480:768])
                    cur = (b, g, fyn, tall)
                else:
                    cur = None

                # interleave batch-1 lhsT build into early iterations
                if k == 2:
                    emit_psall_memset(1, 0, 768, nc.vector)
                    emit_psall_memset(1, 768, 1536, nc.gpsimd)
                elif k == 3:
                    emit_psall_memset(1, 1536, 2304, nc.vector)
                    emit_psall_memset(1, 2304, 3072, nc.gpsimd)
                elif k in (4, 5, 6, 7):
                    emit_psall_copy(1, k - 4, [nc.vector, nc.gpsimd][k % 2])

                if prev is not None:
                    pb, pg, pfy, ptall = prev
                    for yt in range(2):
                        fyv = pfy[:, 128 * yt:128 * yt + 128]
                        nc.tensor.matmul(a01t[yt][:], fyv, ptall[:, 0:512],
                                         start=(pg == 0), stop=(pg == NG - 1))
                        nc.tensor.matmul(a2t[yt][:], fyv, ptall[:, 512:768],
                                         start=(pg == 0), stop=(pg == NG - 1))
                    if pg == NG - 1:
                        ob01 = [wp.tile([128, 512], f32, name=f'ob01_{yt}',
                                        tag=f'ob01_{yt}', bufs=1)
                                for yt in range(2)]
                        ob2 = wp.tile([128, 512], f32, name='ob2', tag='ob2',
                                      bufs=1)
                        nc.scalar.copy(ob01[0][:], a01t[0][:])
                        nc.vector.tensor_copy(ob01[1][:], a01t[1][:])
                        nc.scalar.copy(ob2[:, 0:256], a2t[0][:])
                        nc.vector.tensor_copy(ob2[:, 256:512], a2t[1][:])
                        qs = [nc.sync, nc.scalar, nc.gpsimd,
                              nc.sync, nc.scalar, nc.gpsimd]
                        for yt in range(2):
                            qs[3 * yt].dma_start(
                                y_out[pb, 0, 128 * yt:128 * (yt + 1), :],
                                ob01[yt][:, 0:256])
                            qs[3 * yt + 1].dma_start(
                                y_out[pb, 1, 128 * yt:128 * (yt + 1), :],
                                ob01[yt][:, 256:512])
                            qs[3 * yt + 2].dma_start(
                                y_out[pb, 2, 128 * yt:128 * (yt + 1), :],
                                ob2[:, 256 * yt:256 * (yt + 1)])
                prev = cur

    _split_multi_waits(nc)
    _PROGRAM = nc
    return nc


def _make_in_maps(brushes: np.ndarray, patches: np.ndarray):
    import ml_dtypes
    brushes = np.asarray(brushes, dtype=np.float32)
    patches = np.asarray(patches, dtype=np.float32)
    id4 = np.eye(4, dtype=np.float32)
    nn = np.arange(N)
    # mask4[n, 32j+q'] = 1 where j == n % 4, for every q'
    mask4 = np.zeros((N, 128), dtype=np.float32)
    for n in range(N):
        j = n % 4
        mask4[n, 32 * j:32 * (j + 1)] = 1.0
    mask16 = np.zeros((N, NG), dtype=np.float32)
    mask16[nn, nn // 4] = 1.0
    qp = (np.arange(128) % 32).astype(np.float32).reshape(128, 1)
    in_maps = []
    for k in range(NCORES):
        bsl = brushes[BLOC * k: BLOC * (k + 1)]        # [2, 64, 2]
        g_in = np.ascontiguousarray(
            bsl.transpose(0, 2, 1).reshape(4, N))       # rows b0x,b0y,b1x,b1y
        psl = patches[BLOC * k: BLOC * (k + 1)]         # [2, 64, 3, 32, 32]
        pr = psl.reshape(BLOC, NG, 4, C, PH, PW)[..., ::-1, ::-1]
        # -> [b, j, q', g, c, p'] -> [b, 128, NG*C*PH]
        pt = np.ascontiguousarray(pr.transpose(0, 2, 5, 1, 3, 4)).reshape(
            BLOC, 128, NG * C * PH).astype(ml_dtypes.bfloat16)
        in_maps.append({'g_in': g_in, 'pt_in': pt, 'id4': id4,
                        'mask4': mask4, 'mask16': mask16, 'qp_in': qp})
    return in_maps


def kernel(brushes: np.ndarray, patches: np.ndarray) -> np.ndarray:
    from concourse.bass_utils import run_bass_kernel_spmd

    nc = _build_program()
    in_maps = _make_in_maps(brushes, patches)
    res = run_bass_kernel_spmd(nc, in_maps, list(range(NCORES)))
    out = np.concatenate([res.results[k]['y_out'] for k in range(NCORES)],
                         axis=0)
    return out


# revision 18
# speedup vs baseline: 5.7212x; 5.7212x over previous
"""BrushStroke splat kernel for 8 trn2 NeuronCores.

out[b,c,y,x] = mean_n sum_{p,q} Fy[b,n,y,p] Fx[b,n,x,q] patches[b,n,c,p,q]
with Fx/Fy separable Gaussian filter banks (sigma=0.1) normalized over a
padded spatial axis.

Strategy (per core, 2 batches of 64 strokes), v3 — no DMA gathers, no
gpsimd in the steady state:
 - Per group of 4 strokes, one Derivative_Erf activation per axis
   evaluates (2/sqrt(pi)) * exp(-((t + q') - (g + 31.5))^2 / (2 s^2))
   on a [128(j,q'), 288] iota tile using a per-partition bias built once
   via a one-hot matmul (the 2/sqrt(pi) factor cancels in normalizers).
 - All filter normalizers are precomputed once: E rows per stroke
   [128, 319] -> cumsum scan -> window sums W -> reciprocal -> remapped
   to the per-group [(j,q'), (b,g)] layout with a masked one-hot matmul.
 - MM1 per group: 3 bf16 matmuls (block-diagonal patch lhsT) into one
   PSUM span [128, 768]; a single DVE drain rescales by 1/Wy into bf16.
 - MM2 per group: 4 bf16 matmuls (2 y-tiles x {c0c1 merged, c2}) chained
   over the 16 groups into 4 single-bank PSUM accumulators; unnormalized
   Fy rows are the stationary. MM2 for group g is emitted one iteration
   late so the tensor engine never waits on the drain.
Batch-parallel across cores; no collectives.
"""
import sys, types
import numpy as np

IMAGE = 256
PAD = 16
EPS = 1e-7
SIGMA2 = 2.0 * 0.1 ** 2
B, N, C, PH, PW = 16, 64, 3, 32, 32
NCORES = 8
BLOC = B // NCORES          # 2 batches per core
NG = N // 4                 # 16 groups of 4 strokes
W288 = IMAGE + 2 * PAD      # padded spatial axis length
SCL = (1.0 / SIGMA2) ** 0.5  # derf(SCL*t + SCL*b) ~ exp(-(t+b)^2/SIGMA2)
CX = PW / 2 - 0.5 + PAD      # 31.5
CY = PW / 2 - 0.4 + PAD      # 31.6


def _install_patches():
    if 'antenv.axon_hooks' not in sys.modules:
        mod = types.ModuleType('antenv.axon_hooks')
        mod._hook = None
        mod.set_axon_ntff_profile_hook = lambda h: setattr(mod, '_hook', h)
        mod.get_axon_ntff_profile_hook = lambda: mod._hook
        sys.modules['antenv.axon_hooks'] = mod
        try:
            from trn_agent_boot.trn_boot import _ntff_profile_via_ctypes
            hook = _ntff_profile_via_ctypes('/opt/axon/libaxon_pjrt.so')
            if hook is not None:
                mod.set_axon_ntff_profile_hook(hook)
        except Exception:
            pass

    import concourse.tile as tile
    import concourse.bass_utils as bass_utils
    from concourse.vector_clock import ScopedClock

    bass_utils.upload_artifacts = lambda tmpdir: 'local://' + tmpdir

    if getattr(tile.TileContext._drain_and_barrier, '_patched', False):
        return

    def _drain_and_barrier(self, tick_clock, wait_clock):
        nc = self.nc
        drain_inst = nc.sync.drain()
        wait_clock.add_sem_waits(
            drain_inst.ins, ScopedClock({None: tick_clock.global_clock}))
        si = drain_inst.ins.sync_info
        waits = list(si.on_wait or [])
        si.on_wait = []
        for w in waits:
            nop = nc.sync.nop()
            nop.ins.sync_info = type(si)(on_wait=[w], on_update=[])
        nc.all_engine_barrier()
        popped = nc._tile_sem_poison_stack.pop()
        assert popped is self._sem_poison
        nc.clear_and_free_semaphores(list(self.sems.allocated().values()))
        nc.all_engine_barrier()

    _drain_and_barrier._patched = True
    tile.TileContext._drain_and_barrier = _drain_and_barrier


def _split_multi_waits(nc):
    """This walrus accepts at most one sync wait per instruction; hoist
    extras onto same-engine NoOps inserted just before."""
    import bass_rust
    n_new = [0]

    def fresh_nop(engine, wait, si_type):
        n_new[0] += 1
        nop = bass_rust.InstNoOp(name=f'I-waitsplit-{n_new[0]}', ins=[], outs=[])
        nop.engine = engine
        nop.sync_info = si_type(on_wait=[wait], on_update=[])
        return nop

    for fn in nc.m.functions:
        for blk in fn.blocks:
            insts = blk.instructions
            i = 0
            while i < len(insts):
                inst = insts[i]
                si = inst.sync_info
                if si is not None and si.on_wait and len(si.on_wait) > 1:
                    waits = list(si.on_wait)
                    si.on_wait = [waits[-1]]
                    for k, w in enumerate(waits[:-1]):
                        insts.insert(i + k, fresh_nop(inst.engine, w, type(si)))
                    i += len(waits) - 1
                i += 1


_PROGRAM = None


def _build_program():
    global _PROGRAM
    if _PROGRAM is not None:
        return _PROGRAM
    _install_patches()
    import concourse.bass as bass
    import concourse.tile as tile
    from concourse import mybir
    from bass_rust import AP

    f32 = mybir.dt.float32
    bf16 = mybir.dt.bfloat16
    AF = mybir.ActivationFunctionType
    AX = mybir.AxisListType
    ALU = mybir.AluOpType
    MUL, SUB = ALU.mult, ALU.subtract

    nc = bass.Bass('TRN2', target_bir_lowering=False, debug=False,
                   num_devices=NCORES)
    g_in = nc.declare_dram_parameter('g_in', [4, N], f32, isOutput=False)
    g2_in = nc.declare_dram_parameter('g2_in', [2, 2 * N], f32,
                                      isOutput=False)
    pt_in = nc.declare_dram_parameter('pt_in', [BLOC, 128, NG * C * PH], bf16,
                                      isOutput=False)
    id4 = nc.declare_dram_parameter('id4', [4, 4], f32, isOutput=False)
    mask4 = nc.declare_dram_parameter('mask4', [N, 128], f32, isOutput=False)
    mask16 = nc.declare_dram_parameter('mask16', [N, NG], f32, isOutput=False)
    mask4f = nc.declare_dram_parameter('mask4f', [128, 128], f32,
                                       isOutput=False)
    mask32 = nc.declare_dram_parameter('mask32', [128, 32], f32,
                                       isOutput=False)
    qp_in = nc.declare_dram_parameter('qp_in', [128, 1], f32, isOutput=False)
    y_out = nc.declare_dram_parameter('y_out', [BLOC, C, IMAGE, IMAGE], f32,
                                      isOutput=True)

    with tile.TileContext(nc) as tc:
        with tc.tile_pool(name='glob', bufs=1) as gp, \
             tc.tile_pool(name='work', bufs=1) as wp, \
             tc.tile_pool(name='ps1', bufs=2, space='PSUM') as ps1, \
             tc.tile_pool(name='ps2', bufs=1, space='PSUM') as ps2:
            # accumulators: one PSUM bank per chain (interleaved
            # accumulation groups sharing a bank corrupt each other).
            # Preamble PSUM results squat in their unused columns.
            a01t = [ps2.tile([128, 512], f32, name=f'a01_{yt}')
                    for yt in range(2)]
            a2t = [ps2.tile([128, 256], f32, name=f'a2_{yt}')
                   for yt in range(2)]

            # ---- input DMAs, spread across the 3 DMA-capable queues ----
            bc = gp.tile([4, N], f32)
            nc.sync.dma_start(bc[:], g_in[:])
            g2t = gp.tile([2, 2 * N], f32)
            nc.sync.dma_start(g2t[:], g2_in[:])
            idt = gp.tile([4, 4], f32)
            nc.sync.dma_start(idt[:], id4[:])
            m4 = gp.tile([N, 128], f32)
            nc.scalar.dma_start(m4[:], mask4[:])
            m16 = gp.tile([N, NG], f32)
            nc.scalar.dma_start(m16[:], mask16[:])
            qp = gp.tile([128, 1], f32)
            nc.scalar.dma_start(qp[:], qp_in[:])
            m4f = gp.tile([128, 128], f32)
            nc.scalar.dma_start(m4f[:], mask4f[:])
            m32 = gp.tile([128, 32], f32)
            nc.scalar.dma_start(m32[:], mask32[:])
            ptc = []
            for b in range(BLOC):
                t_ = gp.tile([128, NG * C * PH], bf16, name=f'ptc{b}')
                [nc.gpsimd, nc.sync][b].dma_start(t_[:], pt_in[b])
                ptc.append(t_)

            # ---- iota ramp 0..319 (gpsimd, preamble only) ----
            it = gp.tile([128, 320], f32)
            nc.gpsimd.iota(it[:], pattern=[[1, 320]], base=0,
                           channel_multiplier=0,
                           allow_small_or_imprecise_dtypes=True)

            # ---- block-diagonal patch lhsT (zeros + copies) ----
            ps_all = [gp.tile([128, 128 * C * NG], bf16, name=f'psall{b}')
                      for b in range(BLOC)]

            def emit_psall_memset(b, lo, hi, eng):
                v = ps_all[b].bitcast(f32)
                eng.memset(v[:, lo:hi], 0.0)

            def emit_psall_copy(b, j, eng):
                dst0 = ps_all[b][32 * j:32 * j + 1, 32 * j:32 * j + 1]
                dst = AP(ps_all[b].tensor, dst0.offset,
                         [[128 * C * NG, 32], [128 * C, NG],
                          [128, C], [1, PH]])
                src0 = ptc[b][32 * j:32 * j + 1, 0:1]
                srcap = AP(ptc[b].tensor, src0.offset,
                           [[NG * C * PH, 32], [C * PH, NG],
                            [PH, C], [1, PH]])
                if eng is nc.scalar:
                    eng.copy(dst, srcap)
                else:
                    eng.tensor_copy(dst, srcap)

            emit_psall_memset(0, 0, 1536, nc.vector)
            emit_psall_memset(0, 1536, 3072, nc.gpsimd)

            # ---- brush normalization (grouped layout) -> BiasAll ----
            mn = gp.tile([4, 1], f32)
            mx = gp.tile([4, 1], f32)
            nc.vector.tensor_reduce(mn[:], bc[:], axis=AX.X, op=ALU.min)
            nc.vector.reduce_max(mx[:], bc[:], axis=AX.X)
            rng = gp.tile([4, 1], f32)
            nc.vector.tensor_sub(rng[:], mx[:], mn[:])
            nc.vector.tensor_scalar_add(rng[:], rng[:], EPS)
            inv = gp.tile([4, 1], f32)
            nc.vector.reciprocal(inv[:], rng[:])
            nc.vector.tensor_scalar_mul(inv[:], inv[:], float(IMAGE))
            gn = gp.tile([4, N], f32)
            nc.vector.tensor_scalar_sub(gn[:], bc[:], mn[:])
            nc.vector.tensor_scalar_mul(gn[:], gn[:], inv[:])

            # transpose [4,N] -> [N,4]; psB/transpose squat in a2t[0]
            pre = a2t[0]
            tp_ps = pre[0:N, 64:68]
            nc.tensor.transpose(tp_ps, gn[:], idt[:])
            tp = gp.tile([N, 4], f32)
            nc.vector.tensor_copy(tp[:], tp_ps)

            negC4 = gp.tile([N, 4], f32)
            for k in range(4):
                CC = CX if k % 2 == 0 else CY
                nc.vector.tensor_scalar(negC4[:, k:k + 1], tp[:, k:k + 1],
                                        -SCL, SCL * CC, MUL, SUB)
            rhsA = gp.tile([N, 64], f32)
            for k in range(4):
                nc.vector.tensor_scalar_mul(rhsA[:, 16 * k:16 * (k + 1)],
                                            m16[:], negC4[:, k:k + 1])
            # BiasAll[32j+q', 16k+g] = SCL*(q' - g_{4g+j,k} - C)
            psB = pre[:, 0:64]
            nc.tensor.matmul(psB, m4[:], rhsA[:], start=True, stop=True)
            BiasAll = gp.tile([128, 64], f32)
            nc.vector.tensor_scalar_add(BiasAll[:], psB, qp[:])

            # ---- normalizers: E rows -> scan -> W -> 1/W -> remap ----
            # stroke-major normalization of g2 (cols: b0 strokes | b1)
            gn2 = gp.tile([2, 2 * N], f32)
            for h in range(2):
                sl = slice(N * h, N * (h + 1))
                mn2 = gp.tile([2, 1], f32, name=f'mn2_{h}')
                mx2 = gp.tile([2, 1], f32, name=f'mx2_{h}')
                nc.vector.tensor_reduce(mn2[:], g2t[:, sl], axis=AX.X,
                                        op=ALU.min)
                nc.vector.reduce_max(mx2[:], g2t[:, sl], axis=AX.X)
                rg2 = gp.tile([2, 1], f32, name=f'rg2_{h}')
                nc.vector.tensor_sub(rg2[:], mx2[:], mn2[:])
                nc.vector.tensor_scalar_add(rg2[:], rg2[:], EPS)
                iv2h = gp.tile([2, 1], f32, name=f'iv2h_{h}')
                nc.vector.reciprocal(iv2h[:], rg2[:])
                nc.vector.tensor_scalar_mul(iv2h[:], iv2h[:], float(IMAGE))
                nc.vector.tensor_scalar_sub(gn2[:, sl], g2t[:, sl], mn2[:])
                nc.vector.tensor_scalar_mul(gn2[:, sl], gn2[:, sl], iv2h[:])
            # transpose [2, 128] -> [128, 2]; result squats in a2t[1]
            tpE = a2t[1][:, 0:2]
            nc.tensor.transpose(tpE, gn2[:], idt[0:2, 0:2])
            biasE = gp.tile([128, 2], f32)
            nc.vector.tensor_scalar(biasE[:, 0:1], tpE[:, 0:1],
                                    -SCL, SCL * CX, MUL, SUB)
            nc.vector.tensor_scalar(biasE[:, 1:2], tpE[:, 1:2],
                                    -SCL, SCL * CY, MUL, SUB)

            ivall = []
            for ax in range(2):
                E = gp.tile([128, 320], f32, name=f'E{ax}')
                nc.vector.memset(E[:, 0:1], 0.0)
                nc.scalar.activation(E[:, 1:320], it[:, 0:319],
                                     AF.Derivative_Erf,
                                     bias=biasE[:, ax:ax + 1], scale=SCL)
                CS = gp.tile([128, 320], f32, name=f'CS{ax}')
                nc.vector.tensor_tensor_scan(CS[:], E[:], E[:], 0.0,
                                             ALU.add, ALU.bypass)
                W = gp.tile([128, 32], f32, name=f'W{ax}')
                nc.vector.tensor_tensor(W[:], CS[:, 288:320], CS[:, 0:32],
                                        op=SUB)
                if ax == 0:
                    nc.vector.tensor_scalar_mul(W[:], W[:], float(N))
                Winv = gp.tile([128, 32], f32, name=f'Winv{ax}')
                nc.vector.reciprocal(Winv[:], W[:])
                # masked remap: IvAll[32j+q', 16b+g] = Winv[64b+4g+j, q']
                Wm = gp.tile([128, 128], f32, name=f'Wm{ax}')
                wv0 = Winv[0:1, 0:1]
                wtiled = AP(Winv.tensor, wv0.offset,
                            [[32, 128], [0, 4], [1, 32]])
                nc.vector.tensor_tensor(Wm[:], m4f[:], wtiled, op=MUL)
                iv_ps = a01t[ax][:, 0:32]
                nc.tensor.matmul(iv_ps, Wm[:], m32[:], start=True, stop=True)
                ivA = gp.tile([128, 32], f32, name=f'ivA{ax}')
                nc.vector.tensor_copy(ivA[:], iv_ps)
                ivall.append(ivA)
            ivallx, ivally = ivall

            emit_psall_copy(0, 0, nc.scalar)
            emit_psall_copy(0, 1, nc.vector)
            emit_psall_copy(0, 2, nc.scalar)
            emit_psall_copy(0, 3, nc.vector)

            # ---- main loop: one-group software pipeline ----
            prev = None
            NTOT = BLOC * NG
            for k in range(NTOT + 1):
                if k < NTOT:
                    b, g = divmod(k, NG)
                    colx, coly = 32 * b + g, 32 * b + 16 + g
                    civ = 16 * b + g
                    fx = wp.tile([128, W288], bf16, name='fx', tag='fx',
                                 bufs=2)
                    nc.scalar.activation(fx[:], it[:, 0:W288],
                                         AF.Derivative_Erf,
                                         bias=BiasAll[:, colx:colx + 1],
                                         scale=SCL)
                    fy = wp.tile([128, W288], bf16, name='fy', tag='fy',
                                 bufs=3)
                    nc.scalar.activation(fy[:], it[:, 0:W288],
                                         AF.Derivative_Erf,
                                         bias=BiasAll[:, coly:coly + 1],
                                         scale=SCL)
                    fxn = wp.tile([128, IMAGE], bf16, name='fxn', tag='fxn',
                                  bufs=2)
                    nc.vector.tensor_scalar_mul(fxn[:],
                                                fx[:, PAD:PAD + IMAGE],
                                                ivallx[:, civ:civ + 1])
                    # MM1: 3 channels into one PSUM span
                    pfull = ps1.tile([128, 768], f32, name='pfull',
                                     tag='pfull')
                    for c in range(C):
                        nc.tensor.matmul(
                            pfull[:, 256 * c:256 * (c + 1)],
                            ps_all[b][:, 384 * g + 128 * c:
                                      384 * g + 128 * (c + 1)],
                            fxn[:], start=True, stop=True)
                    # single DVE drain, rescaled by 1/Wy
                    tall = wp.tile([128, 768], bf16, name='tall', tag='tall',
                                   bufs=2)
                    nc.vector.tensor_scalar_mul(tall[:], pfull[:],
                                                ivally[:, civ:civ + 1])
                    cur = (b, g, fy, tall)
                else:
                    cur = None

                # batch-1 lhsT build on gpsimd during early iterations
                if k == 0:
                    emit_psall_memset(1, 0, 1536, nc.gpsimd)
                elif k == 1:
                    emit_psall_memset(1, 1536, 3072, nc.gpsimd)
                elif k in (2, 3, 4, 5):
                    emit_psall_copy(1, k - 2, nc.gpsimd)

                if prev is not None:
                    pb, pg, pfy, ptall = prev
                    for yt in range(2):
                        fyv = pfy[:, PAD + 128 * yt:PAD + 128 * yt + 128]
                        nc.tensor.matmul(a01t[yt][:], fyv, ptall[:, 0:512],
                                         start=(pg == 0), stop=(pg == NG - 1))
                        nc.tensor.matmul(a2t[yt][:], fyv, ptall[:, 512:768],
                                         start=(pg == 0), stop=(pg == NG - 1))
                    if pg == NG - 1:
                        ob01 = [wp.tile([128, 512], f32, name=f'ob01_{yt}',
                                        tag=f'ob01_{yt}', bufs=1)
                                for yt in range(2)]
                        ob2 = wp.tile([128, 512], f32, name='ob2', tag='ob2',
                                      bufs=1)
                        nc.scalar.copy(ob01[0][:], a01t[0][:])
                        nc.vector.tensor_copy(ob01[1][:], a01t[1][:])
                        nc.scalar.copy(ob2[:, 0:256], a2t[0][:])
                        nc.vector.tensor_copy(ob2[:, 256:512], a2t[1][:])
                        qs = [nc.sync, nc.scalar, nc.gpsimd,
                              nc.sync, nc.scalar, nc.gpsimd]
                        for yt in range(2):
                            qs[3 * yt].dma_start(
                                y_out[pb, 0, 128 * yt:128 * (yt + 1), :],
                                ob01[yt][:, 0:256])
                            qs[3 * yt + 1].dma_start(
                                y_out[pb, 1, 128 * yt:128 * (yt + 1), :],
                                ob01[yt][:, 256:512])
                            qs[3 * yt + 2].dma_start(
                                y_out[pb, 2, 128 * yt:128 * (yt + 1), :],
                                ob2[:, 256 * yt:256 * (yt + 1)])
                prev = cur

    _split_multi_waits(nc)
    _PROGRAM = nc
    return nc


def _make_in_maps(brushes: np.ndarray, patches: np.ndarray):
    import ml_dtypes
    brushes = np.asarray(brushes, dtype=np.float32)
    patches = np.asarray(patches, dtype=np.float32)
    id4 = np.eye(4, dtype=np.float32)
    nn = np.arange(N)
    # mask4[n, 32j+q'] = 1 where j == n % 4, for every q'
    mask4 = np.zeros((N, 128), dtype=np.float32)
    for n in range(N):
        j = n % 4
        mask4[n, 32 * j:32 * (j + 1)] = 1.0
    mask16 = np.zeros((N, NG), dtype=np.float32)
    mask16[nn, nn // 4] = 1.0
    mask4f = np.zeros((128, 128), dtype=np.float32)
    for n in range(128):
        j = n % 4
        mask4f[n, 32 * j:32 * (j + 1)] = 1.0
    mask32 = np.zeros((128, 32), dtype=np.float32)
    mask32[np.arange(128), np.arange(128) // 4] = 1.0
    qp = (SCL * (np.arange(128) % 32).astype(np.float32)).reshape(128, 1)
    in_maps = []
    for k in range(NCORES):
        bsl = brushes[BLOC * k: BLOC * (k + 1)]        # [2, 64, 2]
        g_in = np.ascontiguousarray(
            bsl.transpose(0, 2, 1).reshape(4, N))       # rows b0x,b0y,b1x,b1y
        # [2, 128]: row0 = x coords (b0 strokes | b1), row1 = y coords
        g2 = np.ascontiguousarray(
            bsl.transpose(2, 0, 1).reshape(2, 2 * N))
        psl = patches[BLOC * k: BLOC * (k + 1)]         # [2, 64, 3, 32, 32]
        pr = psl.reshape(BLOC, NG, 4, C, PH, PW)[..., ::-1, ::-1]
        # -> [b, j, q', g, c, p'] -> [b, 128, NG*C*PH]
        pt = np.ascontiguousarray(pr.transpose(0, 2, 5, 1, 3, 4)).reshape(
            BLOC, 128, NG * C * PH).astype(ml_dtypes.bfloat16)
        in_maps.append({'g_in': g_in, 'g2_in': g2, 'pt_in': pt, 'id4': id4,
                        'mask4': mask4, 'mask16': mask16, 'mask4f': mask4f,
                        'mask32': mask32, 'qp_in': qp})
    return in_maps


def kernel(brushes: np.ndarray, patches: np.ndarray) -> np.ndarray:
    from concourse.bass_utils import run_bass_kernel_spmd

    nc = _build_program()
    in_maps = _make_in_maps(brushes, patches)
    res = run_bass_kernel_spmd(nc, in_maps, list(range(NCORES)))
    out = np.concatenate([res.results[k]['y_out'] for k in range(NCORES)],
                         axis=0)
    return out


# revision 22
# speedup vs baseline: 6.2488x; 1.0922x over previous
"""BrushStroke splat kernel for 8 trn2 NeuronCores.

out[b,c,y,x] = mean_n sum_{p,q} Fy[b,n,y,p] Fx[b,n,x,q] patches[b,n,c,p,q]
with Fx/Fy separable Gaussian filter banks (sigma=0.1) normalized over a
padded spatial axis.

Strategy (per core, 2 batches of 64 strokes), v3 — no DMA gathers, no
gpsimd in the steady state:
 - Per group of 4 strokes, one Derivative_Erf activation per axis
   evaluates (2/sqrt(pi)) * exp(-((t + q') - (g + 31.5))^2 / (2 s^2))
   on a [128(j,q'), 288] iota tile using a per-partition bias built once
   via a one-hot matmul (the 2/sqrt(pi) factor cancels in normalizers).
 - All filter normalizers are precomputed once: E rows per stroke
   [128, 319] -> cumsum scan -> window sums W -> reciprocal -> remapped
   to the per-group [(j,q'), (b,g)] layout with a masked one-hot matmul.
 - MM1 per group: 3 bf16 matmuls (block-diagonal patch lhsT) into one
   PSUM span [128, 768]; a single DVE drain rescales by 1/Wy into bf16.
 - MM2 per group: 4 bf16 matmuls (2 y-tiles x {c0c1 merged, c2}) chained
   over the 16 groups into 4 single-bank PSUM accumulators; unnormalized
   Fy rows are the stationary. MM2 for group g is emitted one iteration
   late so the tensor engine never waits on the drain.
Batch-parallel across cores; no collectives.
"""
import sys, types
import numpy as np

IMAGE = 256
PAD = 16
EPS = 1e-7
SIGMA2 = 2.0 * 0.1 ** 2
B, N, C, PH, PW = 16, 64, 3, 32, 32
NCORES = 8
BLOC = B // NCORES          # 2 batches per core
NG = N // 4                 # 16 groups of 4 strokes
W288 = IMAGE + 2 * PAD      # padded spatial axis length
SCL = (1.0 / SIGMA2) ** 0.5  # derf(SCL*t + SCL*b) ~ exp(-(t+b)^2/SIGMA2)
CX = PW / 2 - 0.5 + PAD      # 31.5
CY = PW / 2 - 0.4 + PAD      # 31.6


def _install_patches():
    if 'antenv.axon_hooks' not in sys.modules:
        mod = types.ModuleType('antenv.axon_hooks')
        mod._hook = None
        mod.set_axon_ntff_profile_hook = lambda h: setattr(mod, '_hook', h)
        mod.get_axon_ntff_profile_hook = lambda: mod._hook
        sys.modules['antenv.axon_hooks'] = mod
        try:
            from trn_agent_boot.trn_boot import _ntff_profile_via_ctypes
            hook = _ntff_profile_via_ctypes('/opt/axon/libaxon_pjrt.so')
            if hook is not None:
                mod.set_axon_ntff_profile_hook(hook)
        except Exception:
            pass

    import concourse.tile as tile
    import concourse.bass_utils as bass_utils
    from concourse.vector_clock import ScopedClock

    bass_utils.upload_artifacts = lambda tmpdir: 'local://' + tmpdir

    if getattr(tile.TileContext._drain_and_barrier, '_patched', False):
        return

    def _drain_and_barrier(self, tick_clock, wait_clock):
        nc = self.nc
        drain_inst = nc.sync.drain()
        wait_clock.add_sem_waits(
            drain_inst.ins, ScopedClock({None: tick_clock.global_clock}))
        si = drain_inst.ins.sync_info
        waits = list(si.on_wait or [])
        si.on_wait = []
        for w in waits:
            nop = nc.sync.nop()
            nop.ins.sync_info = type(si)(on_wait=[w], on_update=[])
        nc.all_engine_barrier()
        popped = nc._tile_sem_poison_stack.pop()
        assert popped is self._sem_poison
        nc.clear_and_free_semaphores(list(self.sems.allocated().values()))
        nc.all_engine_barrier()

    _drain_and_barrier._patched = True
    tile.TileContext._drain_and_barrier = _drain_and_barrier


def _split_multi_waits(nc):
    """This walrus accepts at most one sync wait per instruction; hoist
    extras onto same-engine NoOps inserted just before."""
    import bass_rust
    n_new = [0]

    def fresh_nop(engine, wait, si_type):
        n_new[0] += 1
        nop = bass_rust.InstNoOp(name=f'I-waitsplit-{n_new[0]}', ins=[], outs=[])
        nop.engine = engine
        nop.sync_info = si_type(on_wait=[wait], on_update=[])
        return nop

    for fn in nc.m.functions:
        for blk in fn.blocks:
            insts = blk.instructions
            i = 0
            while i < len(insts):
                inst = insts[i]
                si = inst.sync_info
                if si is not None and si.on_wait and len(si.on_wait) > 1:
                    waits = list(si.on_wait)
                    si.on_wait = [waits[-1]]
                    for k, w in enumerate(waits[:-1]):
                        insts.insert(i + k, fresh_nop(inst.engine, w, type(si)))
                    i += len(waits) - 1
                i += 1


_PROGRAM = None


def _build_program():
    global _PROGRAM
    if _PROGRAM is not None:
        return _PROGRAM
    _install_patches()
    import concourse.bass as bass
    import concourse.tile as tile
    from concourse import mybir
    from bass_rust import AP

    f32 = mybir.dt.float32
    bf16 = mybir.dt.bfloat16
    AF = mybir.ActivationFunctionType
    AX = mybir.AxisListType
    ALU = mybir.AluOpType
    MUL, SUB = ALU.mult, ALU.subtract

    nc = bass.Bass('TRN2', target_bir_lowering=False, debug=False,
                   num_devices=NCORES)
    g_in = nc.declare_dram_parameter('g_in', [4, N], f32, isOutput=False)
    g2_in = nc.declare_dram_parameter('g2_in', [2, 2 * N], f32,
                                      isOutput=False)
    pt_in = nc.declare_dram_parameter('pt_in', [BLOC, 128, NG * C * PH], bf16,
                                      isOutput=False)
    id4 = nc.declare_dram_parameter('id4', [4, 4], f32, isOutput=False)
    mask4 = nc.declare_dram_parameter('mask4', [N, 128], f32, isOutput=False)
    mask16 = nc.declare_dram_parameter('mask16', [N, NG], f32, isOutput=False)
    mask4f = nc.declare_dram_parameter('mask4f', [128, 128], f32,
                                       isOutput=False)
    mask32 = nc.declare_dram_parameter('mask32', [128, 32], f32,
                                       isOutput=False)
    qp_in = nc.declare_dram_parameter('qp_in', [128, 1], f32, isOutput=False)
    y_out = nc.declare_dram_parameter('y_out', [BLOC, C, IMAGE, IMAGE], f32,
                                      isOutput=True)

    with tile.TileContext(nc) as tc:
        with tc.tile_pool(name='glob', bufs=1) as gp, \
             tc.tile_pool(name='work', bufs=1) as wp, \
             tc.tile_pool(name='ps1', bufs=2, space='PSUM') as ps1, \
             tc.tile_pool(name='ps2', bufs=1, space='PSUM') as ps2:
            # accumulators: one PSUM bank per chain (interleaved
            # accumulation groups sharing a bank corrupt each other).
            # Preamble PSUM results squat in their unused columns.
            a01t = [ps2.tile([128, 512], f32, name=f'a01_{yt}')
                    for yt in range(2)]
            a2t = [ps2.tile([128, 256], f32, name=f'a2_{yt}')
                   for yt in range(2)]

            # ---- input DMAs, spread across the 3 DMA-capable queues ----
            bc = gp.tile([4, N], f32)
            nc.sync.dma_start(bc[:], g_in[:])
            g2t = gp.tile([2, 2 * N], f32)
            nc.sync.dma_start(g2t[:], g2_in[:])
            idt = gp.tile([4, 4], f32)
            nc.sync.dma_start(idt[:], id4[:])
            m4 = gp.tile([N, 128], f32)
            nc.scalar.dma_start(m4[:], mask4[:])
            m16 = gp.tile([N, NG], f32)
            nc.scalar.dma_start(m16[:], mask16[:])
            qp = gp.tile([128, 1], f32)
            nc.scalar.dma_start(qp[:], qp_in[:])
            m4f = gp.tile([128, 128], f32)
            nc.scalar.dma_start(m4f[:], mask4f[:])
            m32 = gp.tile([128, 32], f32)
            nc.scalar.dma_start(m32[:], mask32[:])
            ptc = []
            for b in range(BLOC):
                t_ = gp.tile([128, NG * C * PH], bf16, name=f'ptc{b}')
                [nc.gpsimd, nc.sync][b].dma_start(t_[:], pt_in[b])
                ptc.append(t_)

            # ---- iota ramp 0..319 (gpsimd, preamble only) ----
            it = gp.tile([128, 320], f32)
            nc.gpsimd.iota(it[:], pattern=[[1, 320]], base=0,
                           channel_multiplier=0,
                           allow_small_or_imprecise_dtypes=True)

            # ---- block-diagonal patch lhsT (zeros + copies) ----
            ps_all = [gp.tile([128, 128 * C * NG], bf16, name=f'psall{b}')
                      for b in range(BLOC)]

            def emit_psall_memset(b, lo, hi, eng):
                v = ps_all[b].bitcast(f32)
                eng.memset(v[:, lo:hi], 0.0)

            def emit_psall_copy(b, j, eng):
                dst0 = ps_all[b][32 * j:32 * j + 1, 32 * j:32 * j + 1]
                dst = AP(ps_all[b].tensor, dst0.offset,
                         [[128 * C * NG, 32], [128 * C, NG],
                          [128, C], [1, PH]])
                src0 = ptc[b][32 * j:32 * j + 1, 0:1]
                srcap = AP(ptc[b].tensor, src0.offset,
                           [[NG * C * PH, 32], [C * PH, NG],
                            [PH, C], [1, PH]])
                if eng is nc.scalar:
                    eng.copy(dst, srcap)
                else:
                    eng.tensor_copy(dst, srcap)

            emit_psall_memset(0, 0, 1536, nc.vector)
            emit_psall_memset(0, 1536, 3072, nc.gpsimd)

            # ---- brush normalization (grouped layout) -> BiasAll ----
            mn = gp.tile([4, 1], f32)
            mx = gp.tile([4, 1], f32)
            nc.vector.tensor_reduce(mn[:], bc[:], axis=AX.X, op=ALU.min)
            nc.vector.reduce_max(mx[:], bc[:], axis=AX.X)
            rng = gp.tile([4, 1], f32)
            nc.vector.tensor_sub(rng[:], mx[:], mn[:])
            nc.vector.tensor_scalar_add(rng[:], rng[:], EPS)
            inv = gp.tile([4, 1], f32)
            nc.vector.reciprocal(inv[:], rng[:])
            nc.vector.tensor_scalar_mul(inv[:], inv[:], float(IMAGE))
            gn = gp.tile([4, N], f32)
            nc.vector.tensor_scalar_sub(gn[:], bc[:], mn[:])
            nc.vector.tensor_scalar_mul(gn[:], gn[:], inv[:])

            # transpose [4,N] -> [N,4]; psB/transpose squat in a2t[0]
            pre = a2t[0]
            tp_ps = pre[0:N, 64:68]
            nc.tensor.transpose(tp_ps, gn[:], idt[:])
            tp = gp.tile([N, 4], f32)
            nc.vector.tensor_copy(tp[:], tp_ps)

            negC4 = gp.tile([N, 4], f32)
            for k in range(4):
                CC = CX if k % 2 == 0 else CY
                nc.vector.tensor_scalar(negC4[:, k:k + 1], tp[:, k:k + 1],
                                        -SCL, SCL * CC, MUL, SUB)
            rhsA = gp.tile([N, 64], f32)
            for k in range(4):
                nc.vector.tensor_scalar_mul(rhsA[:, 16 * k:16 * (k + 1)],
                                            m16[:], negC4[:, k:k + 1])
            # BiasAll[32j+q', 16k+g] = SCL*(q' - g_{4g+j,k} - C)
            psB = pre[:, 0:64]
            nc.tensor.matmul(psB, m4[:], rhsA[:], start=True, stop=True)
            BiasAll = gp.tile([128, 64], f32)
            nc.vector.tensor_scalar_add(BiasAll[:], psB, qp[:])

            # ---- normalizers: E rows -> scan -> W -> 1/W -> remap ----
            # stroke-major normalization of g2 (cols: b0 strokes | b1);
            # per-half min/max via a [2(h),64] folded reduce view
            gn2 = gp.tile([2, 2 * N], f32)
            mn2 = gp.tile([2, 2], f32)
            mx2 = gp.tile([2, 2], f32)
            g2v = AP(g2t.tensor, g2t[0:1, 0:1].offset,
                     [[2 * N, 2], [N, 2], [1, N]])
            nc.vector.tensor_reduce(mn2[:], g2v, axis=AX.X, op=ALU.min)
            nc.vector.tensor_reduce(mx2[:], g2v, axis=AX.X, op=ALU.max)
            rg2 = gp.tile([2, 2], f32)
            nc.vector.tensor_tensor(rg2[:], mx2[:], mn2[:], op=SUB)
            nc.vector.tensor_scalar_add(rg2[:], rg2[:], EPS)
            iv2h = gp.tile([2, 2], f32)
            nc.vector.reciprocal(iv2h[:], rg2[:])
            nc.vector.tensor_scalar_mul(iv2h[:], iv2h[:], float(IMAGE))
            mn2b = AP(mn2.tensor, mn2[0:1, 0:1].offset,
                      [[2, 2], [1, 2], [0, N]])
            iv2b = AP(iv2h.tensor, iv2h[0:1, 0:1].offset,
                      [[2, 2], [1, 2], [0, N]])
            nc.vector.tensor_tensor(gn2[:], g2t[:], mn2b, op=SUB)
            nc.vector.tensor_tensor(gn2[:], gn2[:], iv2b, op=MUL)
            # transpose [2, 128] -> [128, 2]; result squats in a2t[1]
            tpE = a2t[1][:, 0:2]
            nc.tensor.transpose(tpE, gn2[:], idt[0:2, 0:2])
            biasE = gp.tile([128, 2], f32)
            nc.vector.tensor_scalar(biasE[:, 0:1], tpE[:, 0:1],
                                    -SCL, SCL * CX, MUL, SUB)
            nc.vector.tensor_scalar(biasE[:, 1:2], tpE[:, 1:2],
                                    -SCL, SCL * CY, MUL, SUB)

            # E rows for both axes in one tile [x-block | y-block]
            E = gp.tile([128, 640], f32)
            ez = AP(E.tensor, E[0:1, 0:1].offset, [[640, 128], [320, 2]])
            nc.vector.memset(ez, 0.0)
            nc.scalar.activation(E[:, 1:320], it[:, 0:319],
                                 AF.Derivative_Erf,
                                 bias=biasE[:, 0:1], scale=SCL)
            emit_psall_copy(0, 0, nc.scalar)
            emit_psall_copy(0, 1, nc.vector)
            nc.scalar.activation(E[:, 321:640], it[:, 0:319],
                                 AF.Derivative_Erf,
                                 bias=biasE[:, 1:2], scale=SCL)
            # one scan over both blocks; the x-total cancels in the
            # window subtraction for the y block
            CS = gp.tile([128, 640], f32)
            nc.vector.tensor_tensor_scan(CS[:], E[:], E[:], 0.0,
                                         ALU.add, ALU.bypass)
            Wxy = gp.tile([128, 64], f32)
            hi = AP(CS.tensor, CS[0:1, 288:289].offset,
                    [[640, 128], [320, 2], [1, 32]])
            lo = AP(CS.tensor, CS[0:1, 0:1].offset,
                    [[640, 128], [320, 2], [1, 32]])
            nc.vector.tensor_tensor(Wxy[:], hi, lo, op=SUB)
            nc.vector.tensor_scalar_mul(Wxy[:, 0:32], Wxy[:, 0:32],
                                        float(N))
            # reference adds EPS to the normalizer; in our units the E rows
            # carry 2/sqrt(pi), and the x block also carries the 1/N fold
            KAP = 2.0 / np.pi ** 0.5
            nc.vector.tensor_scalar_add(Wxy[:, 0:32], Wxy[:, 0:32],
                                        float(N) * KAP * EPS)
            nc.vector.tensor_scalar_add(Wxy[:, 32:64], Wxy[:, 32:64],
                                        KAP * EPS)
            Winv = gp.tile([128, 64], f32)
            nc.vector.reciprocal(Winv[:], Wxy[:])
            # masked remap: IvAll[32j+q', (ax,16b+g)] = Winv[64b+4g+j,
            # (ax,q')]; both axes' lhsT built in one op
            Wm = gp.tile([128, 256], f32)
            m4fd = AP(m4f.tensor, m4f[0:1, 0:1].offset,
                      [[128, 128], [0, 2], [1, 128]])
            wtiled = AP(Winv.tensor, Winv[0:1, 0:1].offset,
                        [[64, 128], [32, 2], [0, 4], [1, 32]])
            nc.vector.tensor_tensor(Wm[:], m4fd, wtiled, op=MUL)
            nc.tensor.matmul(a01t[0][:, 0:32], Wm[:, 0:128], m32[:],
                             start=True, stop=True)
            nc.tensor.matmul(a01t[0][:, 32:64], Wm[:, 128:256], m32[:],
                             start=True, stop=True)
            ivA = gp.tile([128, 64], f32)
            nc.vector.tensor_copy(ivA[:], a01t[0][:, 0:64])

            emit_psall_copy(0, 2, nc.scalar)
            emit_psall_copy(0, 3, nc.vector)

            # ---- main loop: one-group software pipeline on every engine
            # (drain and MM2 for group k-1 are emitted during iteration k,
            # after fxn/MM1 of group k, so no in-order stream ever blocks
            # on a cross-engine producer) ----
            prev = None
            NTOT = BLOC * NG
            for k in range(NTOT + 1):
                if k < NTOT:
                    b, g = divmod(k, NG)
                    colx, coly = 32 * b + g, 32 * b + 16 + g
                    civ = 16 * b + g
                    fx = wp.tile([128, W288], bf16, name='fx', tag='fx',
                                 bufs=3)
                    nc.scalar.activation(fx[:], it[:, 0:W288],
                                         AF.Derivative_Erf,
                                         bias=BiasAll[:, colx:colx + 1],
                                         scale=SCL)
                    fy = wp.tile([128, W288], bf16, name='fy', tag='fy',
                                 bufs=4)
                    nc.scalar.activation(fy[:], it[:, 0:W288],
                                         AF.Derivative_Erf,
                                         bias=BiasAll[:, coly:coly + 1],
                                         scale=SCL)
                    fxn = wp.tile([128, IMAGE], bf16, name='fxn', tag='fxn',
                                  bufs=3)
                    nc.vector.tensor_scalar_mul(fxn[:],
                                                fx[:, PAD:PAD + IMAGE],
                                                ivA[:, civ:civ + 1])
                    # MM1: 3 channels into one PSUM span
                    pfull = ps1.tile([128, 768], f32, name='pfull',
                                     tag='pfull')
                    for c in range(C):
                        nc.tensor.matmul(
                            pfull[:, 256 * c:256 * (c + 1)],
                            ps_all[b][:, 384 * g + 128 * c:
                                      384 * g + 128 * (c + 1)],
                            fxn[:], start=True, stop=True)
                    cur = (b, g, fy, pfull, civ)
                else:
                    cur = None

                # batch-1 lhsT build on gpsimd during early iterations
                if k == 0:
                    emit_psall_memset(1, 0, 1536, nc.gpsimd)
                elif k == 1:
                    emit_psall_memset(1, 1536, 3072, nc.gpsimd)
                elif k in (2, 3, 4, 5):
                    emit_psall_copy(1, k - 2, nc.gpsimd)

                if prev is not None:
                    pb, pg, pfy, ppfull, pciv = prev
                    # drain of group k-1 (DVE), rescaled by 1/Wy
                    tall = wp.tile([128, 768], bf16, name='tall', tag='tall',
                                   bufs=3)
                    nc.vector.tensor_scalar_mul(tall[:], ppfull[:],
                                                ivA[:, 32 + pciv:33 + pciv])
                    for yt in range(2):
                        fyv = pfy[:, PAD + 128 * yt:PAD + 128 * yt + 128]
                        nc.tensor.matmul(a01t[yt][:], fyv, tall[:, 0:512],
                                         start=(pg == 0), stop=(pg == NG - 1))
                        nc.tensor.matmul(a2t[yt][:], fyv, tall[:, 512:768],
                                         start=(pg == 0), stop=(pg == NG - 1))
                    if pg == NG - 1:
                        ob01 = [wp.tile([128, 512], f32, name=f'ob01_{yt}',
                                        tag=f'ob01_{yt}', bufs=1)
                                for yt in range(2)]
                        ob2 = wp.tile([128, 512], f32, name='ob2', tag='ob2',
                                      bufs=1)
                        nc.scalar.copy(ob01[0][:], a01t[0][:])
                        nc.vector.tensor_copy(ob01[1][:], a01t[1][:])
                        nc.scalar.copy(ob2[:, 0:256], a2t[0][:])
                        nc.vector.tensor_copy(ob2[:, 256:512], a2t[1][:])
                        qs = [nc.sync, nc.scalar, nc.gpsimd,
                              nc.sync, nc.scalar, nc.gpsimd]
                        for yt in range(2):
                            qs[3 * yt].dma_start(
                                y_out[pb, 0, 128 * yt:128 * (yt + 1), :],
                                ob01[yt][:, 0:256])
                            qs[3 * yt + 1].dma_start(
                                y_out[pb, 1, 128 * yt:128 * (yt + 1), :],
                                ob01[yt][:, 256:512])
                            qs[3 * yt + 2].dma_start(
                                y_out[pb, 2, 128 * yt:128 * (yt + 1), :],
                                ob2[:, 256 * yt:256 * (yt + 1)])
                prev = cur

    _split_multi_waits(nc)
    _PROGRAM = nc
    return nc


def _make_in_maps(brushes: np.ndarray, patches: np.ndarray):
    import ml_dtypes
    brushes = np.asarray(brushes, dtype=np.float32)
    patches = np.asarray(patches, dtype=np.float32)
    id4 = np.eye(4, dtype=np.float32)
    nn = np.arange(N)
    # mask4[n, 32j+q'] = 1 where j == n % 4, for every q'
    mask4 = np.zeros((N, 128), dtype=np.float32)
    for n in range(N):
        j = n % 4
        mask4[n, 32 * j:32 * (j + 1)] = 1.0
    mask16 = np.zeros((N, NG), dtype=np.float32)
    mask16[nn, nn // 4] = 1.0
    mask4f = np.zeros((128, 128), dtype=np.float32)
    for n in range(128):
        j = n % 4
        mask4f[n, 32 * j:32 * (j + 1)] = 1.0
    mask32 = np.zeros((128, 32), dtype=np.float32)
    mask32[np.arange(128), np.arange(128) // 4] = 1.0
    qp = (SCL * (np.arange(128) % 32).astype(np.float32)).reshape(128, 1)
    in_maps = []
    for k in range(NCORES):
        bsl = brushes[BLOC * k: BLOC * (k + 1)]        # [2, 64, 2]
        g_in = np.ascontiguousarray(
            bsl.transpose(0, 2, 1).reshape(4, N))       # rows b0x,b0y,b1x,b1y
        # [2, 128]: row0 = x coords (b0 strokes | b1), row1 = y coords
        g2 = np.ascontiguousarray(
            bsl.transpose(2, 0, 1).reshape(2, 2 * N))
        psl = patches[BLOC * k: BLOC * (k + 1)]         # [2, 64, 3, 32, 32]
        pr = psl.reshape(BLOC, NG, 4, C, PH, PW)[..., ::-1, ::-1]
        # -> [b, j, q', g, c, p'] -> [b, 128, NG*C*PH]
        pt = np.ascontiguousarray(pr.transpose(0, 2, 5, 1, 3, 4)).reshape(
            BLOC, 128, NG * C * PH).astype(ml_dtypes.bfloat16)
        in_maps.append({'g_in': g_in, 'g2_in': g2, 'pt_in': pt, 'id4': id4,
                        'mask4': mask4, 'mask16': mask16, 'mask4f': mask4f,
                        'mask32': mask32, 'qp_in': qp})
    return in_maps


def kernel(brushes: np.ndarray, patches: np.ndarray) -> np.ndarray:
    from concourse.bass_utils import run_bass_kernel_spmd

    nc = _build_program()
    in_maps = _make_in_maps(brushes, patches)
    res = run_bass_kernel_spmd(nc, in_maps, list(range(NCORES)))
    out = np.concatenate([res.results[k]['y_out'] for k in range(NCORES)],
                         axis=0)
    return out


# revision 28
# speedup vs baseline: 6.5881x; 1.0543x over previous
"""BrushStroke splat kernel for 8 trn2 NeuronCores.

out[b,c,y,x] = mean_n sum_{p,q} Fy[b,n,y,p] Fx[b,n,x,q] patches[b,n,c,p,q]
with Fx/Fy separable Gaussian filter banks (sigma=0.1) normalized over a
padded spatial axis.

Strategy (per core, 2 batches of 64 strokes), v3 — no DMA gathers, no
gpsimd in the steady state:
 - Per group of 4 strokes, one Derivative_Erf activation per axis
   evaluates (2/sqrt(pi)) * exp(-((t + q') - (g + 31.5))^2 / (2 s^2))
   on a [128(j,q'), 288] iota tile using a per-partition bias built once
   via a one-hot matmul (the 2/sqrt(pi) factor cancels in normalizers).
 - All filter normalizers are precomputed once: E rows per stroke
   [128, 319] -> cumsum scan -> window sums W -> reciprocal -> remapped
   to the per-group [(j,q'), (b,g)] layout with a masked one-hot matmul.
 - MM1 per group: 3 bf16 matmuls (block-diagonal patch lhsT) into one
   PSUM span [128, 768]; a single DVE drain rescales by 1/Wy into bf16.
 - MM2 per group: 4 bf16 matmuls (2 y-tiles x {c0c1 merged, c2}) chained
   over the 16 groups into 4 single-bank PSUM accumulators; unnormalized
   Fy rows are the stationary. MM2 for group g is emitted one iteration
   late so the tensor engine never waits on the drain.
Batch-parallel across cores; no collectives.
"""
import sys, types
import numpy as np

IMAGE = 256
PAD = 16
EPS = 1e-7
SIGMA2 = 2.0 * 0.1 ** 2
B, N, C, PH, PW = 16, 64, 3, 32, 32
NCORES = 8
BLOC = B // NCORES          # 2 batches per core
NG = N // 4                 # 16 groups of 4 strokes
W288 = IMAGE + 2 * PAD      # padded spatial axis length
SCL = (1.0 / SIGMA2) ** 0.5  # derf(SCL*t + SCL*b) ~ exp(-(t+b)^2/SIGMA2)
CX = PW / 2 - 0.5 + PAD      # 31.5
CY = PW / 2 - 0.4 + PAD      # 31.6


def _install_patches():
    if 'antenv.axon_hooks' not in sys.modules:
        mod = types.ModuleType('antenv.axon_hooks')
        mod._hook = None
        mod.set_axon_ntff_profile_hook = lambda h: setattr(mod, '_hook', h)
        mod.get_axon_ntff_profile_hook = lambda: mod._hook
        sys.modules['antenv.axon_hooks'] = mod
        try:
            from trn_agent_boot.trn_boot import _ntff_profile_via_ctypes
            hook = _ntff_profile_via_ctypes('/opt/axon/libaxon_pjrt.so')
            if hook is not None:
                mod.set_axon_ntff_profile_hook(hook)
        except Exception:
            pass

    import concourse.tile as tile
    import concourse.bass_utils as bass_utils
    from concourse.vector_clock import ScopedClock

    bass_utils.upload_artifacts = lambda tmpdir: 'local://' + tmpdir

    if getattr(tile.TileContext._drain_and_barrier, '_patched', False):
        return

    def _drain_and_barrier(self, tick_clock, wait_clock):
        nc = self.nc
        drain_inst = nc.sync.drain()
        wait_clock.add_sem_waits(
            drain_inst.ins, ScopedClock({None: tick_clock.global_clock}))
        si = drain_inst.ins.sync_info
        waits = list(si.on_wait or [])
        si.on_wait = []
        for w in waits:
            nop = nc.sync.nop()
            nop.ins.sync_info = type(si)(on_wait=[w], on_update=[])
        nc.all_engine_barrier()
        popped = nc._tile_sem_poison_stack.pop()
        assert popped is self._sem_poison
        nc.clear_and_free_semaphores(list(self.sems.allocated().values()))
        nc.all_engine_barrier()

    _drain_and_barrier._patched = True
    tile.TileContext._drain_and_barrier = _drain_and_barrier


def _split_multi_waits(nc):
    """This walrus accepts at most one sync wait per instruction; hoist
    extras onto same-engine NoOps inserted just before."""
    import bass_rust
    n_new = [0]

    def fresh_nop(engine, wait, si_type):
        n_new[0] += 1
        nop = bass_rust.InstNoOp(name=f'I-waitsplit-{n_new[0]}', ins=[], outs=[])
        nop.engine = engine
        nop.sync_info = si_type(on_wait=[wait], on_update=[])
        return nop

    for fn in nc.m.functions:
        for blk in fn.blocks:
            insts = blk.instructions
            i = 0
            while i < len(insts):
                inst = insts[i]
                si = inst.sync_info
                if si is not None and si.on_wait and len(si.on_wait) > 1:
                    waits = list(si.on_wait)
                    si.on_wait = [waits[-1]]
                    for k, w in enumerate(waits[:-1]):
                        insts.insert(i + k, fresh_nop(inst.engine, w, type(si)))
                    i += len(waits) - 1
                i += 1


_PROGRAM = None


def _build_program():
    global _PROGRAM
    if _PROGRAM is not None:
        return _PROGRAM
    _install_patches()
    import concourse.bass as bass
    import concourse.tile as tile
    from concourse import mybir
    from bass_rust import AP

    f32 = mybir.dt.float32
    bf16 = mybir.dt.bfloat16
    AF = mybir.ActivationFunctionType
    AX = mybir.AxisListType
    ALU = mybir.AluOpType
    MUL, SUB = ALU.mult, ALU.subtract

    nc = bass.Bass('TRN2', target_bir_lowering=False, debug=False,
                   num_devices=NCORES)
    g_in = nc.declare_dram_parameter('g_in', [4, N], f32, isOutput=False)
    g2_in = nc.declare_dram_parameter('g2_in', [2, 2 * N], f32,
                                      isOutput=False)
    pt_in = nc.declare_dram_parameter('pt_in', [BLOC, 128, NG * C * PH], bf16,
                                      isOutput=False)
    id4 = nc.declare_dram_parameter('id4', [4, 4], f32, isOutput=False)
    mask4 = nc.declare_dram_parameter('mask4', [N, 128], f32, isOutput=False)
    mask16 = nc.declare_dram_parameter('mask16', [N, NG], f32, isOutput=False)
    mask4f2 = nc.declare_dram_parameter('mask4f2', [128, 256], f32,
                                        isOutput=False)
    mask32 = nc.declare_dram_parameter('mask32', [128, 32], f32,
                                       isOutput=False)
    qp2_in = nc.declare_dram_parameter('qp2_in', [128, 64], f32,
                                       isOutput=False)
    y_out = nc.declare_dram_parameter('y_out', [BLOC, C, IMAGE, IMAGE], f32,
                                      isOutput=True)

    with tile.TileContext(nc) as tc:
        with tc.tile_pool(name='glob', bufs=1) as gp, \
             tc.tile_pool(name='work', bufs=1) as wp, \
             tc.tile_pool(name='ps1', bufs=2, space='PSUM') as ps1, \
             tc.tile_pool(name='ps2', bufs=1, space='PSUM') as ps2:
            # accumulators: one PSUM bank per chain (interleaved
            # accumulation groups sharing a bank corrupt each other).
            # Preamble PSUM results squat in their unused columns.
            a01t = [ps2.tile([128, 512], f32, name=f'a01_{yt}')
                    for yt in range(2)]
            a2t = [ps2.tile([128, 256], f32, name=f'a2_{yt}')
                   for yt in range(2)]

            # ---- iota ramp 0..319 (gpsimd, preamble only) ----
            it = gp.tile([128, 320], f32)
            nc.gpsimd.iota(it[:], pattern=[[1, 320]], base=0,
                           channel_multiplier=0,
                           allow_small_or_imprecise_dtypes=True)

            # ---- input DMAs, spread across the 3 DMA-capable queues ----
            g2t = gp.tile([2, 2 * N], f32)
            nc.sync.dma_start(g2t[:], g2_in[:])
            idt = gp.tile([4, 4], f32)
            nc.sync.dma_start(idt[:], id4[:])
            bc = gp.tile([4, N], f32)
            nc.sync.dma_start(bc[:], g_in[:])
            m16 = gp.tile([N, NG], f32)
            nc.scalar.dma_start(m16[:], mask16[:])
            m4 = gp.tile([N, 128], f32)
            nc.scalar.dma_start(m4[:], mask4[:])
            qp2 = gp.tile([128, 64], f32)
            nc.scalar.dma_start(qp2[:], qp2_in[:])
            m4f2 = gp.tile([128, 256], f32)
            nc.scalar.dma_start(m4f2[:], mask4f2[:])
            m32 = gp.tile([128, 32], f32)
            nc.scalar.dma_start(m32[:], mask32[:])
            ptc = []
            for b in range(BLOC):
                t_ = gp.tile([128, NG * C * PH], bf16, name=f'ptc{b}')
                [nc.gpsimd, nc.sync][b].dma_start(t_[:], pt_in[b])
                ptc.append(t_)

            # ---- block-diagonal patch lhsT (zeros + copies) ----
            ps_all = [gp.tile([128, 128 * C * NG], bf16, name=f'psall{b}')
                      for b in range(BLOC)]

            def emit_psall_memset(b, lo, hi, eng):
                v = ps_all[b].bitcast(f32)
                eng.memset(v[:, lo:hi], 0.0)

            def emit_psall_copy(b, j, eng):
                dst0 = ps_all[b][32 * j:32 * j + 1, 32 * j:32 * j + 1]
                dst = AP(ps_all[b].tensor, dst0.offset,
                         [[128 * C * NG, 32], [128 * C, NG],
                          [128, C], [1, PH]])
                src0 = ptc[b][32 * j:32 * j + 1, 0:1]
                srcap = AP(ptc[b].tensor, src0.offset,
                           [[NG * C * PH, 32], [C * PH, NG],
                            [PH, C], [1, PH]])
                if eng is nc.scalar:
                    eng.copy(dst, srcap)
                else:
                    eng.tensor_copy(dst, srcap)

            emit_psall_memset(0, 0, 1536, nc.gpsimd)
            emit_psall_memset(0, 1536, 3072, nc.gpsimd)

            # ---- normalizer chain first (it gates the E rows) ----
            # stroke-major normalization of g2 (cols: b0 strokes | b1);
            # per-half min/max via a [2(h),64] folded reduce view
            gn2 = gp.tile([2, 2 * N], f32)
            mn2 = gp.tile([2, 2], f32)
            mx2 = gp.tile([2, 2], f32)
            g2v = AP(g2t.tensor, g2t[0:1, 0:1].offset,
                     [[2 * N, 2], [N, 2], [1, N]])
            nc.vector.tensor_reduce(mn2[:], g2v, axis=AX.X, op=ALU.min)
            nc.vector.tensor_reduce(mx2[:], g2v, axis=AX.X, op=ALU.max)
            rg2 = gp.tile([2, 2], f32)
            nc.vector.tensor_tensor(rg2[:], mx2[:], mn2[:], op=SUB)
            nc.vector.tensor_scalar_add(rg2[:], rg2[:], EPS)
            iv2h = gp.tile([2, 2], f32)
            nc.vector.reciprocal(iv2h[:], rg2[:])
            nc.vector.tensor_scalar_mul(iv2h[:], iv2h[:], float(IMAGE))
            mn2b = AP(mn2.tensor, mn2[0:1, 0:1].offset,
                      [[2, 2], [1, 2], [0, N]])
            iv2b = AP(iv2h.tensor, iv2h[0:1, 0:1].offset,
                      [[2, 2], [1, 2], [0, N]])
            nc.vector.tensor_tensor(gn2[:], g2t[:], mn2b, op=SUB)
            nc.vector.tensor_tensor(gn2[:], gn2[:], iv2b, op=MUL)
            # transpose [2, 128] -> [128, 2]; result squats in a2t[1]
            tpE = a2t[1][:, 0:2]
            nc.tensor.transpose(tpE, gn2[:], idt[0:2, 0:2])
            biasE = gp.tile([128, 2], f32)
            nc.vector.tensor_scalar(biasE[:, 0:1], tpE[:, 0:1],
                                    -SCL, SCL * CX, MUL, SUB)
            nc.vector.tensor_scalar(biasE[:, 1:2], tpE[:, 1:2],
                                    -SCL, SCL * CY, MUL, SUB)

            # ---- brush normalization (grouped layout) -> BiasAll ----
            mn = gp.tile([4, 1], f32)
            mx = gp.tile([4, 1], f32)
            nc.vector.tensor_reduce(mn[:], bc[:], axis=AX.X, op=ALU.min)
            nc.vector.reduce_max(mx[:], bc[:], axis=AX.X)
            rng = gp.tile([4, 1], f32)
            nc.vector.tensor_sub(rng[:], mx[:], mn[:])
            nc.vector.tensor_scalar_add(rng[:], rng[:], EPS)
            inv = gp.tile([4, 1], f32)
            nc.vector.reciprocal(inv[:], rng[:])
            nc.vector.tensor_scalar_mul(inv[:], inv[:], float(IMAGE))
            gn = gp.tile([4, N], f32)
            nc.vector.tensor_scalar_sub(gn[:], bc[:], mn[:])
            nc.vector.tensor_scalar_mul(gn[:], gn[:], inv[:])

            # transpose [4,N] -> [N,4]; psB/transpose squat in a2t[0]
            pre = a2t[0]
            tp_ps = pre[0:N, 64:68]
            nc.tensor.transpose(tp_ps, gn[:], idt[:])
            # rhsA[n, 16k+g] = mask16[n,g] * (-SCL * tp[n,k]) in two ops
            negT = gp.tile([N, 4], f32)
            nc.vector.tensor_scalar_mul(negT[:], tp_ps, -SCL)
            rhsA = gp.tile([N, 64], f32)
            m16d = AP(m16.tensor, m16[0:1, 0:1].offset,
                      [[NG, N], [0, 4], [1, NG]])
            negTd = AP(negT.tensor, negT[0:1, 0:1].offset,
                       [[4, N], [1, 4], [0, NG]])
            nc.vector.tensor_tensor(rhsA[:], m16d, negTd, op=MUL)
            # BiasAll[32j+q', 16k+g] = SCL*(q' - g_{4g+j,k} - C(k));
            # the -SCL*C(k) and SCL*q' terms come in via the qp2 constant
            psB = pre[:, 0:64]
            nc.tensor.matmul(psB, m4[:], rhsA[:], start=True, stop=True)
            BiasAll = gp.tile([128, 64], f32)
            nc.vector.tensor_tensor(BiasAll[:], psB, qp2[:], op=ALU.add)

            # ---- E rows -> scan -> W -> 1/W -> masked remap ----
            E = gp.tile([128, 640], f32)
            ez = AP(E.tensor, E[0:1, 0:1].offset, [[640, 128], [320, 2]])
            nc.vector.memset(ez, 0.0)
            nc.scalar.activation(E[:, 1:320], it[:, 0:319],
                                 AF.Derivative_Erf,
                                 bias=biasE[:, 0:1], scale=SCL)
            nc.scalar.activation(E[:, 321:640], it[:, 0:319],
                                 AF.Derivative_Erf,
                                 bias=biasE[:, 1:2], scale=SCL)
            # one scan over both blocks; the x-total cancels in the
            # window subtraction for the y block
            CS = gp.tile([128, 640], f32)
            nc.vector.tensor_tensor_scan(CS[:], E[:], E[:], 0.0,
                                         ALU.add, ALU.bypass)
            Wxy = gp.tile([128, 64], f32)
            hi = AP(CS.tensor, CS[0:1, 288:289].offset,
                    [[640, 128], [320, 2], [1, 32]])
            lo = AP(CS.tensor, CS[0:1, 0:1].offset,
                    [[640, 128], [320, 2], [1, 32]])
            nc.vector.tensor_tensor(Wxy[:], hi, lo, op=SUB)
            # reference adds EPS to the normalizer (E rows carry 2/sqrt(pi));
            # the 1/N fold for the x block lives in the mask4f2 constant
            nc.vector.tensor_scalar_add(Wxy[:], Wxy[:],
                                        (2.0 / np.pi ** 0.5) * EPS)
            Winv = gp.tile([128, 64], f32)
            nc.vector.reciprocal(Winv[:], Wxy[:])
            # masked remap: IvAll[32j+q', (ax,16b+g)] = Winv[64b+4g+j,
            # (ax,q')] (x scaled by 1/N via mask4f2)
            Wm = gp.tile([128, 256], f32)
            wtiled = AP(Winv.tensor, Winv[0:1, 0:1].offset,
                        [[64, 128], [32, 2], [0, 4], [1, 32]])
            nc.vector.tensor_tensor(Wm[:], m4f2[:], wtiled, op=MUL)
            nc.tensor.matmul(a01t[0][:, 0:32], Wm[:, 0:128], m32[:],
                             start=True, stop=True)
            nc.tensor.matmul(a01t[0][:, 32:64], Wm[:, 128:256], m32[:],
                             start=True, stop=True)
            ivA = gp.tile([128, 64], f32)
            nc.vector.tensor_copy(ivA[:], a01t[0][:, 0:64])

            for j in range(4):
                emit_psall_copy(0, j, nc.scalar)

            # ---- main loop: one-group software pipeline on every engine
            # (drain and MM2 for group k-1 are emitted during iteration k,
            # after fxn/MM1 of group k, so no in-order stream ever blocks
            # on a cross-engine producer) ----
            prev = None
            NTOT = BLOC * NG
            for k in range(NTOT + 1):
                if k < NTOT:
                    b, g = divmod(k, NG)
                    colx, coly = 32 * b + g, 32 * b + 16 + g
                    civ = 16 * b + g
                    fx = wp.tile([128, W288], bf16, name='fx', tag='fx',
                                 bufs=3)
                    nc.scalar.activation(fx[:], it[:, 0:W288],
                                         AF.Derivative_Erf,
                                         bias=BiasAll[:, colx:colx + 1],
                                         scale=SCL)
                    fy = wp.tile([128, W288], bf16, name='fy', tag='fy',
                                 bufs=4)
                    nc.scalar.activation(fy[:], it[:, 0:W288],
                                         AF.Derivative_Erf,
                                         bias=BiasAll[:, coly:coly + 1],
                                         scale=SCL)
                    fxn = wp.tile([128, IMAGE], bf16, name='fxn', tag='fxn',
                                  bufs=3)
                    nc.vector.tensor_scalar_mul(fxn[:],
                                                fx[:, PAD:PAD + IMAGE],
                                                ivA[:, civ:civ + 1])
                    # MM1: 3 channels into one PSUM span
                    pfull = ps1.tile([128, 768], f32, name='pfull',
                                     tag='pfull')
                    for c in range(C):
                        nc.tensor.matmul(
                            pfull[:, 256 * c:256 * (c + 1)],
                            ps_all[b][:, 384 * g + 128 * c:
                                      384 * g + 128 * (c + 1)],
                            fxn[:], start=True, stop=True)
                    cur = (b, g, fy, pfull, civ)
                else:
                    cur = None

                # batch-1 lhsT build on gpsimd during early iterations
                if k == 0:
                    emit_psall_memset(1, 0, 1536, nc.gpsimd)
                elif k == 1:
                    emit_psall_memset(1, 1536, 3072, nc.gpsimd)
                elif k in (2, 3, 4, 5):
                    emit_psall_copy(1, k - 2, nc.gpsimd)

                if prev is not None:
                    pb, pg, pfy, ppfull, pciv = prev
                    # drain of group k-1 (DVE), rescaled by 1/Wy
                    tall = wp.tile([128, 768], bf16, name='tall', tag='tall',
                                   bufs=3)
                    nc.vector.tensor_scalar_mul(tall[:], ppfull[:],
                                                ivA[:, 32 + pciv:33 + pciv])
                    for yt in range(2):
                        fyv = pfy[:, PAD + 128 * yt:PAD + 128 * yt + 128]
                        nc.tensor.matmul(a01t[yt][:], fyv, tall[:, 0:512],
                                         start=(pg == 0), stop=(pg == NG - 1))
                        nc.tensor.matmul(a2t[yt][:], fyv, tall[:, 512:768],
                                         start=(pg == 0), stop=(pg == NG - 1))
                    if pg == NG - 1:
                        ob01 = [wp.tile([128, 512], f32, name=f'ob01_{yt}',
                                        tag=f'ob01_{yt}', bufs=1)
                                for yt in range(2)]
                        ob2 = wp.tile([128, 512], f32, name='ob2', tag='ob2',
                                      bufs=1)
                        nc.scalar.copy(ob01[0][:], a01t[0][:])
                        nc.vector.tensor_copy(ob01[1][:], a01t[1][:])
                        nc.scalar.copy(ob2[:, 0:256], a2t[0][:])
                        nc.vector.tensor_copy(ob2[:, 256:512], a2t[1][:])
                        qs = [nc.sync, nc.scalar, nc.sync,
                              nc.scalar, nc.sync, nc.scalar]
                        for yt in range(2):
                            qs[3 * yt].dma_start(
                                y_out[pb, 0, 128 * yt:128 * (yt + 1), :],
                                ob01[yt][:, 0:256])
                            qs[3 * yt + 1].dma_start(
                                y_out[pb, 1, 128 * yt:128 * (yt + 1), :],
                                ob01[yt][:, 256:512])
                            qs[3 * yt + 2].dma_start(
                                y_out[pb, 2, 128 * yt:128 * (yt + 1), :],
                                ob2[:, 256 * yt:256 * (yt + 1)])
                prev = cur

    _split_multi_waits(nc)
    _PROGRAM = nc
    return nc


def _make_in_maps(brushes: np.ndarray, patches: np.ndarray):
    import ml_dtypes
    brushes = np.asarray(brushes, dtype=np.float32)
    patches = np.asarray(patches, dtype=np.float32)
    id4 = np.eye(4, dtype=np.float32)
    nn = np.arange(N)
    # mask4[n, 32j+q'] = 1 where j == n % 4, for every q'
    mask4 = np.zeros((N, 128), dtype=np.float32)
    for n in range(N):
        j = n % 4
        mask4[n, 32 * j:32 * (j + 1)] = 1.0
    mask16 = np.zeros((N, NG), dtype=np.float32)
    mask16[nn, nn // 4] = 1.0
    mask4f = np.zeros((128, 128), dtype=np.float32)
    for n in range(128):
        j = n % 4
        mask4f[n, 32 * j:32 * (j + 1)] = 1.0
    # x half folds the 1/N mean scale; y half is the plain mask
    mask4f2 = np.concatenate([mask4f / N, mask4f], axis=1)
    mask32 = np.zeros((128, 32), dtype=np.float32)
    mask32[np.arange(128), np.arange(128) // 4] = 1.0
    # qp2[p, 16k+g] = SCL*(q'(p) - C(k)), C = CX for x cols, CY for y
    qprime = (np.arange(128) % 32).astype(np.float32)
    cks = np.array([CX, CY, CX, CY], dtype=np.float32)
    qp2 = SCL * (qprime[:, None] - np.repeat(cks, NG)[None, :])
    in_maps = []
    for k in range(NCORES):
        bsl = brushes[BLOC * k: BLOC * (k + 1)]        # [2, 64, 2]
        g_in = np.ascontiguousarray(
            bsl.transpose(0, 2, 1).reshape(4, N))       # rows b0x,b0y,b1x,b1y
        # [2, 128]: row0 = x coords (b0 strokes | b1), row1 = y coords
        g2 = np.ascontiguousarray(
            bsl.transpose(2, 0, 1).reshape(2, 2 * N))
        psl = patches[BLOC * k: BLOC * (k + 1)]         # [2, 64, 3, 32, 32]
        pr = psl.reshape(BLOC, NG, 4, C, PH, PW)[..., ::-1, ::-1]
        # -> [b, j, q', g, c, p'] -> [b, 128, NG*C*PH]
        pt = np.ascontiguousarray(pr.transpose(0, 2, 5, 1, 3, 4)).reshape(
            BLOC, 128, NG * C * PH).astype(ml_dtypes.bfloat16)
        in_maps.append({'g_in': g_in, 'g2_in': g2, 'pt_in': pt, 'id4': id4,
                        'mask4': mask4, 'mask16': mask16,
                        'mask4f2': mask4f2.astype(np.float32),
                        'mask32': mask32, 'qp2_in': qp2.astype(np.float32)})
    return in_maps


def kernel(brushes: np.ndarray, patches: np.ndarray) -> np.ndarray:
    from concourse.bass_utils import run_bass_kernel_spmd

    nc = _build_program()
    in_maps = _make_in_maps(brushes, patches)
    res = run_bass_kernel_spmd(nc, in_maps, list(range(NCORES)))
    out = np.concatenate([res.results[k]['y_out'] for k in range(NCORES)],
                         axis=0)
    return out
